# revision 1
# baseline (speedup 1.0000x reference)
"""C3DLoss kernel for Trainium2 — 8-core batch-parallel, raw-Bass implementation.

Per core = one batch frame b (tgt pairing partner tb = b^1):
    partial = sum over both terms (same-frame, cross-frame), all 25 shifts
              delta in [-2,2]^2, all pixels p of
        mref(p) * mq(p+delta) * exp(-50*(|xyz_r(p)-xyz_q(p+d)|^2
                                         + |rgb_r(p)-rgb_q(p+d)|^2))
    loss = -(sum of partials) / max(sum(depth_gt_mask), 1)

Device mapping:
  - Host pre-blocks every plane into G=32 W-blocks of width WB with a +-2
    halo in both dims (zero padded).  Partitions = (channel, block); dy/dx
    shifts become pure free-dim offsets, so all 25 shifts read the same
    SBUF tiles.  Host also precomputes the feature planes (xyz = xy1*depth,
    txyz = R*xyz + t, mask channels); that is <2% of the FLOPs.
  - Channels split across two tiles (PE contraction K <= 128):
      A: x, y, z, 20*(1-mq) query-mask channel (ref side 0) -> +400 if masked
      B: r, g, b
    Ref-mask 400*(1-mg) is injected by a third selector matmul.
  - Per shift: DVE subtract (fp32 in, bf16 out) over full haloed rows
    (1 free dim -> single-wait-capable ISA structs), DVE square (bf16 2x),
    selector matmuls reduce channels into a 32-partition PSUM slot
    (4 shifts per 128-partition PSUM bank), ScalarE exp(-50*d2) with fused
    accum_out -> per-partition partial sums; halo columns are skipped by
    the strided matmul rhs.
  - Raw engine programs with explicit semaphores: this toolchain only
    supports one embedded sync-wait per instruction, so every wait is its
    own wait_ge instruction (TileContext emits multi-wait instructions and
    cannot compile here).
"""

import sys

for _p in ("/opt/trn_rl_repo", "/opt/pypackages"):
    if _p not in sys.path:
        sys.path.insert(0, _p)

from contextlib import ExitStack

import numpy as np
import ml_dtypes

import concourse.bass as bass
import concourse.mybir as mybir
from concourse.ap import AP
from concourse.alu_op_type import AluOpType

F32 = mybir.dt.float32
BF16 = mybir.dt.bfloat16
BF_NP = ml_dtypes.bfloat16

R = 2
G = 32           # W-blocks; one shift-slot = 32 partitions (PE quadrant)
CA = 4           # tile A channels: x, y, z, query-mask
CB = 3           # tile B channels: r, g, b
SBATCH = 4       # shift slots per 128-partition PSUM bank
NPSUM = 6        # rotating PSUM banks
NSQ = 8          # rotating sq buffers
MQ_C = 20.0
MR_C = 400.0
EXP_SCALE = -50.0


class Cfg:
    def __init__(self, H=352, W=1216, HS=32):
        assert W % G == 0 and H % HS == 0
        self.H, self.W, self.HS = H, W, HS
        self.WB = W // G
        self.WBH = self.WB + 2 * R
        self.Hp = H + 2 * R
        self.NSLAB = H // HS
        self.NQ = G * self.Hp * self.WBH     # haloed plane elems
        self.QF = (HS + 2 * R) * self.WBH    # query tile free size
        self.SF = HS * self.WBH              # slab tile free size (full width)
        # row-chunks per slab: PSUM bank holds <=512 f32 per partition
        cr = max(1, 512 // self.WB)
        self.rchunks = []
        o = 0
        while o < HS:
            self.rchunks.append((o, min(cr, HS - o)))
            o += cr
        self.slots = [(t, dy, dx) for t in (0, 1)
                      for dy in range(-R, R + 1) for dx in range(-R, R + 1)]
        self.batches = [self.slots[i:i + SBATCH]
                        for i in range(0, len(self.slots), SBATCH)]
        self.NB = len(self.batches)          # 13
        self.NC = len(self.rchunks)          # units per batch
        self.UPS = self.NB * self.NC         # units per slab
        self.n_acc = self.NSLAB * self.UPS


def _apv(t_ap, p0, pcnt, free_dims, free_off=0):
    pstride = t_ap.ap[0][0]
    base = t_ap.offset + p0 * pstride + free_off
    return AP(t_ap.tensor, base, [[pstride, pcnt]] + [list(d) for d in free_dims])


def _dram_ap(handle, offset, dims):
    a = handle[:]
    return AP(a.tensor, a.offset + offset, [list(d) for d in dims])


def make_selA():
    s = np.zeros((CA * G, G), dtype=BF_NP)
    for c in range(CA):
        for g in range(G):
            s[c * G + g, g] = 1
    return s


def make_selB():
    s = np.zeros((CB * G, G), dtype=BF_NP)
    for c in range(CB):
        for g in range(G):
            s[c * G + g, g] = 1
    return s


def make_selvr():
    s = np.zeros((G, SBATCH * G), dtype=BF_NP)
    for t in range(SBATCH):
        for g in range(G):
            s[g, t * G + g] = 1
    return s


def emit(nc: bass.Bass, cfg: Cfg):
    HS, WB, WBH, Hp = cfg.HS, cfg.WB, cfg.WBH, cfg.Hp
    NQ, QF, SF = cfg.NQ, cfg.QF, cfg.SF
    NSLAB, NB, NC, UPS = cfg.NSLAB, cfg.NB, cfg.NC, cfg.UPS
    Act = mybir.ActivationFunctionType

    dp = nc.declare_dram_parameter
    # all planes in blocked+haloed geometry, flat [*, NQ] f32 (VR bf16)
    qa_d = dp("qa_d", [2, CA, NQ], F32, isOutput=False)   # query xyz+Vq per term
    ra_d = dp("ra_d", [2, CA, NQ], F32, isOutput=False)   # ref xyz+0 per term
    qb_d = dp("qb_d", [CB, NQ], F32, isOutput=False)      # query rgb (frame b)
    rbt_d = dp("rbt_d", [CB, NQ], F32, isOutput=False)    # ref rgb (frame tb)
    vr_d = dp("vr_d", [2, NQ], BF16, isOutput=False)      # 400*(1-mg) per term
    selA_d = dp("selA_d", [CA * G, G], BF16, isOutput=False)
    selB_d = dp("selB_d", [CB * G, G], BF16, isOutput=False)
    selvr_d = dp("selvr_d", [G, SBATCH * G], BF16, isOutput=False)
    out_d = dp("out_d", [128, 1], F32, isOutput=True)
    dbg_d = dp("dbg_d", [128, cfg.n_acc], F32, isOutput=True)

    LD = 8            # load DMAs per slab
    NCONST = 3        # constant DMAs at start

    def unit(s, b, c):
        return s * UPS + b * NC + c

    with ExitStack() as ex:
        E = ex.enter_context
        # SBUF buffers (double-buffered per slab phase)
        qa_s = [[E(nc.sbuf_tensor(f"qa{t}{p}", [CA * G, QF + 4], F32))
                 for p in range(2)] for t in range(2)]
        ra_s = [[E(nc.sbuf_tensor(f"ra{t}{p}", [CA * G, SF], F32))
                 for p in range(2)] for t in range(2)]
        qb_s = [E(nc.sbuf_tensor(f"qb{p}", [CB * G, QF + 4], F32))
                for p in range(2)]
        rbt_s = [E(nc.sbuf_tensor(f"rbt{p}", [CB * G, SF], F32))
                 for p in range(2)]
        vr_s = [[E(nc.sbuf_tensor(f"vr{t}{p}", [G, SF], BF16))
                 for p in range(2)] for t in range(2)]
        da_s = E(nc.sbuf_tensor("da", [CA * G, SF], BF16))
        db_s = E(nc.sbuf_tensor("db", [CB * G, SF], BF16))
        sqa_s = [E(nc.sbuf_tensor(f"sqa{i}", [CA * G, SF], BF16))
                 for i in range(NSQ)]
        sqb_s = [E(nc.sbuf_tensor(f"sqb{i}", [CB * G, SF], BF16))
                 for i in range(NSQ)]
        kt_s = [E(nc.sbuf_tensor(f"kt{i}", [128, 512], BF16))
                for i in range(2)]
        acc_s = E(nc.sbuf_tensor("acc", [128, cfg.n_acc], F32))
        res_s = E(nc.sbuf_tensor("res", [128, 1], F32))
        selA_s = E(nc.sbuf_tensor("selA", [CA * G, G], BF16))
        selB_s = E(nc.sbuf_tensor("selB", [CB * G, G], BF16))
        selvr_s = E(nc.sbuf_tensor("selvr", [G, SBATCH * G], BF16))
        ps_s = [E(nc.psum_tensor(f"ps{i}", [128, 512], F32))
                for i in range(NPSUM)]

        sL = E(nc.semaphore("sL"))   # misc DMA completions (+16 each)
        sLC = E(nc.semaphore("sLC"))  # constant loads
        sL0 = E(nc.semaphore("sL0"))  # even-slab loads
        sL1 = E(nc.semaphore("sL1"))  # odd-slab loads
        sG = E(nc.semaphore("sG"))   # gpsimd memset done
        sV = E(nc.semaphore("sV"))   # DVE slots done
        sP = E(nc.semaphore("sP"))   # PE units done
        sA = E(nc.semaphore("sA"))   # ACT units done
        blk = E(nc.Block())

        @blk.gpsimd
        def _(gp):
            gp.memset(acc_s.ap(), 0.0)
            gp.memset(res_s.ap(), 0.0)
            for t in range(2):
                for p in range(2):
                    gp.memset(qa_s[t][p].ap(), 0.0)
            for p in range(2):
                gp.memset(qb_s[p].ap(), 0.0)
            gp.drain()
            gp.sem_inc(sG, 8)

        @blk.sync
        def _(sp):
            sp.dma_start(selA_s[:], selA_d[:]).then_inc(sLC, 16)
            sp.dma_start(selB_s[:], selB_d[:]).then_inc(sLC, 16)
            sp.dma_start(selvr_s[:], selvr_d[:]).then_inc(sLC, 16)
            sp.wait_ge(sG, 8)
            for s in range(NSLAB):
                ph = s % 2
                if s >= 2:
                    sp.wait_ge(sV, 50 * (s - 1))
                    sp.wait_ge(sP, UPS * (s - 1))
                r0 = s * HS
                sLs = sL0 if s % 2 == 0 else sL1
                for t in range(2):
                    sp.dma_start(
                        _apv(qa_s[t][ph].ap(), 0, CA * G, [[1, QF]], 2),
                        _dram_ap(qa_d, t * CA * NQ + r0 * WBH,
                                 [[NQ, CA], [Hp * WBH, G], [1, QF]])
                    ).then_inc(sLs, 16)
                    sp.dma_start(
                        ra_s[t][ph].ap(),
                        _dram_ap(ra_d, t * CA * NQ + (r0 + 2) * WBH,
                                 [[NQ, CA], [Hp * WBH, G], [1, SF]])
                    ).then_inc(sLs, 16)
                    sp.dma_start(
                        vr_s[t][ph].ap(),
                        _dram_ap(vr_d, t * NQ + (r0 + 2) * WBH,
                                 [[Hp * WBH, G], [1, SF]])
                    ).then_inc(sLs, 16)
                sp.dma_start(
                    _apv(qb_s[ph].ap(), 0, CB * G, [[1, QF]], 2),
                    _dram_ap(qb_d, r0 * WBH,
                             [[NQ, CB], [Hp * WBH, G], [1, QF]])
                ).then_inc(sLs, 16)
                sp.dma_start(
                    rbt_s[ph].ap(),
                    _dram_ap(rbt_d, (r0 + 2) * WBH,
                             [[NQ, CB], [Hp * WBH, G], [1, SF]])
                ).then_inc(sLs, 16)
            # final output
            sp.wait_ge(sV, 50 * NSLAB + 1)
            sp.dma_start(out_d[:], res_s.ap()).then_inc(sL, 16)
            sp.dma_start(dbg_d[:], acc_s.ap()).then_inc(sL, 16)

        @blk.vector
        def _(ve):
            J = 0
            ve.wait_ge(sLC, 16 * NCONST)
            for s in range(NSLAB):
                ph = s % 2
                sLs = sL0 if s % 2 == 0 else sL1
                ve.wait_ge(sLs, 16 * LD * (s // 2 + 1))
                for j5, (t, dy, dx) in enumerate(cfg.slots):
                    if J >= NSQ:
                        Jo = J - NSQ
                        oldb = (Jo // 50) * NB + (Jo % 50) // SBATCH
                        ve.wait_ge(sP, NC * (oldb + 1))
                    qoff = 2 + (2 + dy) * WBH + dx
                    nc.vector.tensor_tensor(
                        da_s.ap(), ra_s[t][ph].ap(),
                        _apv(qa_s[t][ph].ap(), 0, CA * G, [[1, SF]], qoff),
                        AluOpType.subtract)
                    nc.vector.tensor_mul(sqa_s[J % NSQ].ap(), da_s.ap(), da_s.ap())
                    rb_ap = (_apv(qb_s[ph].ap(), 0, CB * G, [[1, SF]], 2 + 2 * WBH)
                             if t == 0 else rbt_s[ph].ap())
                    nc.vector.tensor_tensor(
                        db_s.ap(), rb_ap,
                        _apv(qb_s[ph].ap(), 0, CB * G, [[1, SF]], qoff),
                        AluOpType.subtract)
                    nc.vector.tensor_mul(
                        sqb_s[J % NSQ].ap(), db_s.ap(), db_s.ap())
                    ve.drain()
                    ve.sem_inc(sV, 1)
                    J += 1
            # final reduction of acc columns
            ve.wait_ge(sA, NSLAB * UPS)
            nc.vector.tensor_reduce(
                res_s.ap(), acc_s.ap(), axis=mybir.AxisListType.X,
                op=AluOpType.add)
            ve.drain()
            ve.sem_inc(sV, 1)

        @blk.tensor
        def _(pe):
            pe.wait_ge(sLC, 16 * NCONST)
            for s in range(NSLAB):
                ph = s % 2
                for b, bslots in enumerate(cfg.batches):
                    for c in range(NC):
                        u = unit(s, b, c)
                        if u >= NPSUM:
                            pe.wait_ge(sA, u - NPSUM + 1)
                    runs = []
                    for j, (t, _, _) in enumerate(bslots):
                        if runs and runs[-1][0] == t:
                            runs[-1][2] += 1
                        else:
                            runs.append([t, j, 1])
                    pieces = []
                    for (t, soff, scnt) in runs:
                        x, end = soff, soff + scnt
                        while x < end:
                            for size in (4, 2, 1):
                                if x % size == 0 and x + size <= end:
                                    pieces.append((t, x, size))
                                    x += size
                                    break
                    for j, (t, dy, dx) in enumerate(bslots):
                        Jg = s * 50 + b * SBATCH + j
                        pe.wait_ge(sV, Jg + 1)
                        for c, (ro, nr) in enumerate(cfg.rchunks):
                            u = unit(s, b, c)
                            pt = ps_s[u % NPSUM]
                            cn = nr * WB
                            nc.tensor.matmul(
                                pt[G * j:G * (j + 1), :cn], selA_s[:],
                                _apv(sqa_s[Jg % NSQ].ap(), 0, CA * G,
                                     [[WBH, nr], [1, WB]], ro * WBH + 2),
                                start=True, stop=False, skip_group_check=True,
                                tile_position=(0, G * j))
                            nc.tensor.matmul(
                                pt[G * j:G * (j + 1), :cn], selB_s[:],
                                _apv(sqb_s[Jg % NSQ].ap(), 0, CB * G,
                                     [[WBH, nr], [1, WB]], ro * WBH + 2),
                                start=False, stop=False, skip_group_check=True,
                                tile_position=(0, G * j))
                    for c, (ro, nr) in enumerate(cfg.rchunks):
                        u = unit(s, b, c)
                        pt = ps_s[u % NPSUM]
                        cn = nr * WB
                        for pi, (t, soff, scnt) in enumerate(pieces):
                            mm = nc.tensor.matmul(
                                pt[G * soff:G * (soff + scnt), :cn],
                                selvr_s[:, :G * scnt],
                                _apv(vr_s[t][ph].ap(), 0, G,
                                     [[WBH, nr], [1, WB]], ro * WBH + 2),
                                start=False, stop=True, skip_group_check=True,
                                tile_position=(0, G * soff))
                            if pi == len(pieces) - 1:
                                pe.drain()
                                pe.sem_inc(sP, 1)

        @blk.scalar
        def _(ac):
            ac.wait_ge(sG, 1)
            for s in range(NSLAB):
                for b in range(NB):
                    pb = G * len(cfg.batches[b])
                    for c, (ro, nr) in enumerate(cfg.rchunks):
                        u = unit(s, b, c)
                        ac.wait_ge(sP, u + 1)
                        cn = nr * WB
                        nc.scalar.activation(
                            kt_s[u % 2][:pb, :cn], ps_s[u % NPSUM][:pb, :cn],
                            Act.Exp, scale=EXP_SCALE,
                            accum_out=acc_s[:pb, u:u + 1])
                        ac.drain()
                        ac.sem_inc(sA, 1)
    return nc


# ---------------- host side ----------------

def _block_q(plane, cfg):
    """[H, W] -> flat blocked+haloed [G*Hp*WBH], zero-padded borders."""
    p = np.zeros((cfg.Hp, cfg.W + 2 * R), dtype=np.float32)
    p[R:R + cfg.H, R:R + cfg.W] = plane
    out = np.empty((G, cfg.Hp, cfg.WBH), dtype=np.float32)
    for g in range(G):
        out[g] = p[:, g * cfg.WB:g * cfg.WB + cfg.WBH]
    return np.ascontiguousarray(out).reshape(-1)


def host_precompute(rgb, depth, depth_gt, depth_mask, depth_gt_mask,
                    xy1_grid, Ts, cfg, b):
    tb = b ^ 1
    xy1 = np.asarray(xy1_grid[b], np.float32)
    dep = np.asarray(depth[b, 0], np.float32)
    dgt_b = np.asarray(depth_gt[b, 0], np.float32)
    dgt_t = np.asarray(depth_gt[tb, 0], np.float32)
    mp = np.asarray(depth_mask[b, 0], np.float32)
    mg_b = np.asarray(depth_gt_mask[b, 0], np.float32)
    mg_t = np.asarray(depth_gt_mask[tb, 0], np.float32)

    xyz_p = xy1 * dep
    T21 = (np.linalg.inv(np.asarray(Ts[tb], np.float64)) @
           np.asarray(Ts[b], np.float64)).astype(np.float32)
    Rm, tv = T21[:3, :3], T21[:3, 3]
    txyz = np.einsum('ij,jhw->ihw', Rm, xyz_p).astype(np.float32) \
        + tv[:, None, None].astype(np.float32)
    pos = (txyz[2] > 0).astype(np.float32) * mp

    qa = np.empty((2, CA, cfg.NQ), np.float32)
    ra = np.empty((2, CA, cfg.NQ), np.float32)
    for c in range(3):
        qa[0, c] = _block_q(xyz_p[c], cfg)
        qa[1, c] = _block_q(txyz[c], cfg)
        ra[0, c] = _block_q(xy1[c] * dgt_b, cfg)
        ra[1, c] = _block_q(xy1[c] * dgt_t, cfg)
    qa[0, 3] = MQ_C * (1.0 - _block_q(mp, cfg))
    qa[1, 3] = MQ_C * (1.0 - _block_q(pos, cfg))
    ra[:, 3] = 0.0
    qb = np.stack([_block_q(np.asarray(rgb[b, c], np.float32), cfg)
                   for c in range(3)])
    rbt = np.stack([_block_q(np.asarray(rgb[tb, c], np.float32), cfg)
                    for c in range(3)])
    vr = np.stack([MR_C * (1.0 - _block_q(mg_b, cfg)),
                   MR_C * (1.0 - _block_q(mg_t, cfg))]).astype(BF_NP)
    return {"qa_d": qa, "ra_d": ra, "qb_d": qb, "rbt_d": rbt, "vr_d": vr,
            "selA_d": make_selA(), "selB_d": make_selB(),
            "selvr_d": make_selvr()}


def make_in_maps(rgb, depth, depth_gt, depth_mask, depth_gt_mask, xy1_grid, Ts,
                 cfg, n_cores=8):
    return [host_precompute(rgb, depth, depth_gt, depth_mask, depth_gt_mask,
                            xy1_grid, Ts, cfg, b) for b in range(n_cores)]


_CACHED = {}


def _get_nc(cfg_key=(352, 1216, 32)):
    if cfg_key not in _CACHED:
        cfg = Cfg(*cfg_key)
        nc = bass.Bass()
        emit(nc, cfg)
        _CACHED[cfg_key] = (nc, cfg)
    return _CACHED[cfg_key]


def kernel(rgb, depth, depth_gt, depth_mask, depth_gt_mask, xy1_grid, Ts,
           **run_kwargs):
    from concourse.bass_utils import run_bass_kernel_spmd
    nc, cfg = _get_nc()
    maps = make_in_maps(rgb, depth, depth_gt, depth_mask, depth_gt_mask,
                        xy1_grid, Ts, cfg)
    res = run_bass_kernel_spmd(nc, maps, list(range(8)), **run_kwargs)
    total = np.float64(0.0)
    for r in res.results:
        total += np.float64(r["out_d"][:, 0].sum())
    n_gt = max(np.asarray(depth_gt_mask, np.float64).sum(), 1.0)
    loss = -total / n_gt
    kernel.last_results = res
    return np.float32(loss)



# revision 2
# speedup vs baseline: 1.3885x; 1.3885x over previous
"""C3DLoss kernel for Trainium2 — 8-core batch-parallel, raw-Bass implementation.

Per core = one batch frame b (tgt pairing partner tb = b^1):
    partial = sum over both terms (same-frame, cross-frame), all 25 shifts
              delta in [-2,2]^2, all pixels p of
        mref(p) * mq(p+delta) * exp(-50*(|xyz_r(p)-xyz_q(p+d)|^2
                                         + |rgb_r(p)-rgb_q(p+d)|^2))
    loss = -(sum of partials) / max(sum(depth_gt_mask), 1)

v2 design (vs the v1 baseline at kernel_v1_baseline.py):
  - All feature planes stored fp16: DVE tensor_tensor runs in 2x_1p mode
    (2 elem/cycle/lane) when every operand is 2-byte packed.  The
    subtract-then-square order keeps precision (diffs are small; the
    catastrophic-cancellation dot-product form is not usable).
  - Both masks folded into feature channel 3: ra3 = +20*(1-mref),
    qa3 = -20*(1-mq); (ra3 - qa3)^2 is 0 when both masks pass and
    >= 400 otherwise, so exp(-50*...) == 0 exactly.  This removes the
    separate ref-mask matmul pass of v1 entirely (PE: 3 passes -> 2).
  - Compact d/sq tiles (halo columns skipped via strided reads), so DVE,
    ScalarE and PE all stream 1216 useful columns per slot.
  - Work split per slot across three engines: DVE does subA (xyz+mask),
    subB (rgb, even slots only) and sqA; GpSimd does subB on odd slots;
    ScalarE squares the rgb diffs (pairs of slots in one instruction)
    and does the exp+accumulate from PSUM as before.
  - PE per slot: 3 chunk matmuls with selA (128-contraction) accumulating
    into PSUM + 3 with selB (96-contraction) finishing the accumulation.
"""

import sys

for _p in ("/opt/trn_rl_repo", "/opt/pypackages"):
    if _p not in sys.path:
        sys.path.insert(0, _p)

from contextlib import ExitStack

import numpy as np

import concourse.bass as bass
import concourse.mybir as mybir
from concourse.ap import AP
from concourse.alu_op_type import AluOpType

F32 = mybir.dt.float32
F16 = mybir.dt.float16

R = 2
G = 32            # W-blocks; one shift-slot = 32 PSUM partitions
CA = 4            # tile A channels: x, y, z, mask
CB = 3            # tile B channels: r, g, b
SBATCH = 4        # shift slots per 128-partition PSUM bank
NPSUM = 6         # rotating PSUM banks (unit = (batch, chunk))
NSQA = 4          # rotating sqa buffers (per slot)
NDP = 3           # rotating db pair buffers (per 2 slots)
NSQB = 2          # rotating sqb pair buffers (per 2 slots)
MK = 20.0         # mask channel scale; (2*MK)^2 = 1600 >> 1/50
EXP_SCALE = -50.0


class Cfg:
    def __init__(self, H=352, W=1216, HS=32):
        assert W % G == 0 and H % HS == 0
        self.H, self.W, self.HS = H, W, HS
        self.WB = W // G                      # 38
        self.WBH = self.WB + 2 * R            # 42
        self.Hp = H + 2 * R                   # 356
        self.NSLAB = H // HS                  # 11
        self.NQ = G * self.Hp * self.WBH      # haloed plane elems
        self.QF = (HS + 2 * R) * self.WBH     # query tile free size 1512
        self.RF = HS * self.WBH               # ref tile free size 1344
        self.SF = HS * self.WB                # compact slot free size 1216
        # col-chunks of the compact slot (PSUM bank <= 512 f32/partition)
        cr = (512 // self.WB) * self.WB       # 494
        self.chunks = []
        o = 0
        while o < self.SF:
            self.chunks.append((o, min(cr, self.SF - o)))
            o += cr
        self.NC = len(self.chunks)            # 3
        self.slots = [(t, dy, dx) for t in (0, 1)
                      for dy in range(-R, R + 1) for dx in range(-R, R + 1)]
        self.NS = len(self.slots)             # 50
        assert self.NS % 2 == 0
        self.batches = [self.slots[i:i + SBATCH]
                        for i in range(0, self.NS, SBATCH)]
        self.NB = len(self.batches)           # 13
        self.TOTS = self.NSLAB * self.NS      # 550 slots
        self.TOTB = self.NSLAB * self.NB      # 143 batches
        self.n_acc = self.TOTB * self.NC      # 429 acc columns


def _apv(t_ap, p0, pcnt, free_dims, free_off=0):
    pstride = t_ap.ap[0][0]
    base = t_ap.offset + p0 * pstride + free_off
    return AP(t_ap.tensor, base, [[pstride, pcnt]] + [list(d) for d in free_dims])


def _dram_ap(handle, offset, dims):
    a = handle[:]
    return AP(a.tensor, a.offset + offset, [list(d) for d in dims])


def make_selA():
    s = np.zeros((CA * G, G), dtype=np.float16)
    for c in range(CA):
        for g in range(G):
            s[c * G + g, g] = 1
    return s


def make_selB():
    s = np.zeros((CB * G, G), dtype=np.float16)
    for c in range(CB):
        for g in range(G):
            s[c * G + g, g] = 1
    return s


def emit(nc: bass.Bass, cfg: Cfg):
    HS, WB, WBH, Hp = cfg.HS, cfg.WB, cfg.WBH, cfg.Hp
    NQ, QF, RF, SF = cfg.NQ, cfg.QF, cfg.RF, cfg.SF
    NSLAB, NB, NC, NS = cfg.NSLAB, cfg.NB, cfg.NC, cfg.NS
    Act = mybir.ActivationFunctionType
    HpW = Hp * WBH

    dp = nc.declare_dram_parameter
    qa_d = dp("qa_d", [2, CA, NQ], F16, isOutput=False)   # query xyz+mask per term
    ra_d = dp("ra_d", [2, CA, NQ], F16, isOutput=False)   # ref xyz+mask per term
    qb_d = dp("qb_d", [CB, NQ], F16, isOutput=False)      # query rgb (frame b)
    rbt_d = dp("rbt_d", [CB, NQ], F16, isOutput=False)    # ref rgb (frame tb)
    selA_d = dp("selA_d", [CA * G, G], F16, isOutput=False)
    selB_d = dp("selB_d", [CB * G, G], F16, isOutput=False)
    out_d = dp("out_d", [128, 1], F32, isOutput=True)

    LD = 6            # load DMAs per slab
    NCONST = 2

    with ExitStack() as ex:
        E = ex.enter_context
        # double-buffered (per slab phase) input planes, haloed layout
        qa_s = [[E(nc.sbuf_tensor(f"qa{t}{p}", [CA * G, QF], F16))
                 for p in range(2)] for t in range(2)]
        ra_s = [[E(nc.sbuf_tensor(f"ra{t}{p}", [CA * G, RF], F16))
                 for p in range(2)] for t in range(2)]
        qb_s = [E(nc.sbuf_tensor(f"qb{p}", [CB * G, QF], F16)) for p in range(2)]
        rbt_s = [E(nc.sbuf_tensor(f"rbt{p}", [CB * G, RF], F16)) for p in range(2)]
        # work tiles, compact layout
        da_s = E(nc.sbuf_tensor("da", [CA * G, SF], F16))
        db_s = [E(nc.sbuf_tensor(f"db{i}", [CB * G, 2 * SF], F16))
                for i in range(NDP)]
        sqa_s = [E(nc.sbuf_tensor(f"sqa{i}", [CA * G, SF], F16))
                 for i in range(NSQA)]
        sqb_s = [E(nc.sbuf_tensor(f"sqb{i}", [CB * G, 2 * SF], F16))
                 for i in range(NSQB)]
        kt_s = [E(nc.sbuf_tensor(f"kt{i}", [128, 512], F16)) for i in range(2)]
        acc_s = E(nc.sbuf_tensor("acc", [128, cfg.n_acc], F32))
        res_s = E(nc.sbuf_tensor("res", [128, 1], F32))
        selA_s = E(nc.sbuf_tensor("selA", [CA * G, G], F16))
        selB_s = E(nc.sbuf_tensor("selB", [CB * G, G], F16))
        ps_s = [E(nc.psum_tensor(f"ps{i}", [128, 512], F32))
                for i in range(NPSUM)]

        sL = E(nc.semaphore("sL"))    # final output DMA
        sLC = E(nc.semaphore("sLC"))  # constant loads
        sL0 = E(nc.semaphore("sL0"))  # even-slab loads
        sL1 = E(nc.semaphore("sL1"))  # odd-slab loads
        sG = E(nc.semaphore("sG"))    # gpsimd memset done
        sV = E(nc.semaphore("sV"))    # DVE slot done (subA/subB-even/sqA)
        sPl = E(nc.semaphore("sPl"))  # Pool subB-odd done (per odd slot)
        sAq = E(nc.semaphore("sAq"))  # Act sqB pair done (per 2 slots)
        sP = E(nc.semaphore("sP"))    # PE slot done
        sA = E(nc.semaphore("sA"))    # Act exp units done (per unit)
        blk = E(nc.Block())

        # access-pattern builders ------------------------------------------
        def q_ap(tile, pcnt, dy, dx):
            # compact read of the (dy, dx)-shifted query window
            off = (R + dy) * WBH + (R + dx)
            return _apv(tile.ap(), 0, pcnt, [[WBH, HS], [1, WB]], off)

        def r_ap(tile, pcnt):
            # compact read of the ref window (haloed rows already centered)
            return _apv(tile.ap(), 0, pcnt, [[WBH, HS], [1, WB]], R)

        def qb_center_ap(ph):
            # t=0 ref rgb == query rgb plane read at center
            return q_ap(qb_s[ph], CB * G, 0, 0)

        def compact_ap(tile, pcnt, ncols, off=0):
            return _apv(tile.ap(), 0, pcnt, [[1, ncols]], off)

        def rgbref_ap(t, ph):
            return qb_center_ap(ph) if t == 0 else r_ap(rbt_s[ph], CB * G)

        @blk.gpsimd
        def _(gp):
            gp.memset(acc_s.ap(), 0.0)
            gp.memset(res_s.ap(), 0.0)
            gp.drain()
            gp.sem_inc(sG, 1)
            for s in range(NSLAB):
                ph = s % 2
                sLs = sL0 if ph == 0 else sL1
                gp.wait_ge(sLs, 16 * LD * (s // 2 + 1))
                for j5 in range(1, NS, 2):
                    J = s * NS + j5
                    gpi = J // 2
                    t, dy, dx = cfg.slots[j5]
                    if gpi - (NDP - 1) >= 1:
                        gp.wait_ge(sAq, gpi - (NDP - 1))
                    nc.gpsimd.tensor_tensor(
                        _apv(db_s[gpi % NDP].ap(), 0, CB * G,
                             [[WB, HS], [1, WB]], SF),
                        rgbref_ap(t, ph),
                        q_ap(qb_s[ph], CB * G, dy, dx),
                        AluOpType.subtract)
                    gp.drain()
                    gp.sem_inc(sPl, 1)

        @blk.sync
        def _(sp):
            sp.dma_start(selA_s[:], selA_d[:]).then_inc(sLC, 16)
            sp.dma_start(selB_s[:], selB_d[:]).then_inc(sLC, 16)
            for s in range(NSLAB):
                ph = s % 2
                if s >= 2:
                    sp.wait_ge(sV, NS * (s - 1))
                    sp.wait_ge(sPl, (NS // 2) * (s - 1))
                r0 = s * HS
                sLs = sL0 if ph == 0 else sL1
                for t in range(2):
                    sp.dma_start(
                        qa_s[t][ph].ap(),
                        _dram_ap(qa_d, t * CA * NQ + r0 * WBH,
                                 [[NQ, CA], [HpW, G], [1, QF]])
                    ).then_inc(sLs, 16)
                    sp.dma_start(
                        ra_s[t][ph].ap(),
                        _dram_ap(ra_d, t * CA * NQ + (r0 + R) * WBH,
                                 [[NQ, CA], [HpW, G], [1, RF]])
                    ).then_inc(sLs, 16)
                sp.dma_start(
                    qb_s[ph].ap(),
                    _dram_ap(qb_d, r0 * WBH, [[NQ, CB], [HpW, G], [1, QF]])
                ).then_inc(sLs, 16)
                sp.dma_start(
                    rbt_s[ph].ap(),
                    _dram_ap(rbt_d, (r0 + R) * WBH,
                             [[NQ, CB], [HpW, G], [1, RF]])
                ).then_inc(sLs, 16)
            sp.wait_ge(sV, cfg.TOTS + 1)
            sp.dma_start(out_d[:], res_s.ap()).then_inc(sL, 16)

        @blk.vector
        def _(ve):
            for s in range(NSLAB):
                ph = s % 2
                sLs = sL0 if ph == 0 else sL1
                ve.wait_ge(sLs, 16 * LD * (s // 2 + 1))
                for j5 in range(NS):
                    J = s * NS + j5
                    gpi = J // 2
                    t, dy, dx = cfg.slots[j5]
                    if J - NSQA >= 0:
                        ve.wait_ge(sP, J - NSQA + 1)
                    if j5 % 2 == 0 and gpi - NDP >= 0:
                        ve.wait_ge(sAq, gpi - NDP + 1)
                    nc.vector.tensor_tensor(
                        _apv(da_s.ap(), 0, CA * G, [[WB, HS], [1, WB]]),
                        r_ap(ra_s[t][ph], CA * G),
                        q_ap(qa_s[t][ph], CA * G, dy, dx),
                        AluOpType.subtract)
                    if j5 % 2 == 0:
                        nc.vector.tensor_tensor(
                            _apv(db_s[gpi % NDP].ap(), 0, CB * G,
                                 [[WB, HS], [1, WB]]),
                            rgbref_ap(t, ph),
                            q_ap(qb_s[ph], CB * G, dy, dx),
                            AluOpType.subtract)
                    nc.vector.tensor_mul(
                        compact_ap(sqa_s[J % NSQA], CA * G, SF),
                        compact_ap(da_s, CA * G, SF),
                        compact_ap(da_s, CA * G, SF))
                    ve.drain()
                    ve.sem_inc(sV, 1)
            # final reduction of acc columns
            ve.wait_ge(sA, cfg.TOTB * NC)
            nc.vector.tensor_reduce(
                res_s.ap(), acc_s.ap(), axis=mybir.AxisListType.X,
                op=AluOpType.add)
            ve.drain()
            ve.sem_inc(sV, 1)

        @blk.tensor
        def _(pe):
            pe.wait_ge(sLC, 16 * NCONST)
            for s in range(NSLAB):
                for b in range(NB):
                    gb = s * NB + b
                    bslots = cfg.batches[b]
                    if NC * gb - NPSUM >= 0:
                        pe.wait_ge(sA, NC * gb - NPSUM + NC)
                    for jj in range(len(bslots)):
                        J = s * NS + b * SBATCH + jj
                        pe.wait_ge(sV, J + 1)
                        for c, (co, cn) in enumerate(cfg.chunks):
                            u = gb * NC + c
                            nc.tensor.matmul(
                                ps_s[u % NPSUM][G * jj:G * (jj + 1), :cn],
                                selA_s[:],
                                compact_ap(sqa_s[J % NSQA], CA * G, cn, co),
                                start=True, stop=False, skip_group_check=True,
                                tile_position=(0, G * jj))
                        pe.wait_ge(sAq, J // 2 + 1)
                        for c, (co, cn) in enumerate(cfg.chunks):
                            u = gb * NC + c
                            nc.tensor.matmul(
                                ps_s[u % NPSUM][G * jj:G * (jj + 1), :cn],
                                selB_s[:],
                                compact_ap(sqb_s[(J // 2) % NSQB], CB * G, cn,
                                           (J % 2) * SF + co),
                                start=False, stop=True, skip_group_check=True,
                                tile_position=(0, G * jj))
                        pe.drain()
                        pe.sem_inc(sP, 1)

        @blk.scalar
        def _(ac):
            ac.wait_ge(sG, 1)
            for s in range(NSLAB):
                for b in range(NB):
                    gb = s * NB + b
                    L = len(cfg.batches[b])
                    gJ0 = s * NS + b * SBATCH
                    for k in range(L // 2):
                        gpi = gJ0 // 2 + k
                        ac.wait_ge(sV, 2 * gpi + 1)
                        ac.wait_ge(sPl, gpi + 1)
                        if gpi - NSQB >= 0:
                            ac.wait_ge(sP, 2 * (gpi - NSQB) + 2)
                        nc.scalar.activation(
                            compact_ap(sqb_s[gpi % NSQB], CB * G, 2 * SF),
                            compact_ap(db_s[gpi % NDP], CB * G, 2 * SF),
                            Act.Square)
                        ac.drain()
                        ac.sem_inc(sAq, 1)
                    pb = G * L
                    ac.wait_ge(sP, gJ0 + L)
                    for c, (co, cn) in enumerate(cfg.chunks):
                        u = gb * NC + c
                        nc.scalar.activation(
                            kt_s[u % 2][:pb, :cn], ps_s[u % NPSUM][:pb, :cn],
                            Act.Exp, scale=EXP_SCALE,
                            accum_out=acc_s[:pb, u:u + 1])
                    ac.drain()
                    ac.sem_inc(sA, NC)
    return nc


# ---------------- host side ----------------

def _block_q(plane, cfg):
    """[H, W] -> flat blocked+haloed [G*Hp*WBH] fp16, zero-padded borders."""
    p = np.zeros((cfg.Hp, cfg.W + 2 * R), dtype=np.float32)
    p[R:R + cfg.H, R:R + cfg.W] = plane
    out = np.empty((G, cfg.Hp, cfg.WBH), dtype=np.float16)
    for g in range(G):
        out[g] = p[:, g * cfg.WB:g * cfg.WB + cfg.WBH]
    return np.ascontiguousarray(out).reshape(-1)


def host_precompute(rgb, depth, depth_gt, depth_mask, depth_gt_mask,
                    xy1_grid, Ts, cfg, b):
    tb = b ^ 1
    xy1 = np.asarray(xy1_grid[b], np.float32)
    dep = np.asarray(depth[b, 0], np.float32)
    dgt_b = np.asarray(depth_gt[b, 0], np.float32)
    dgt_t = np.asarray(depth_gt[tb, 0], np.float32)
    mp = np.asarray(depth_mask[b, 0], np.float32)
    mg_b = np.asarray(depth_gt_mask[b, 0], np.float32)
    mg_t = np.asarray(depth_gt_mask[tb, 0], np.float32)

    xyz_p = xy1 * dep
    T21 = (np.linalg.inv(np.asarray(Ts[tb], np.float64)) @
           np.asarray(Ts[b], np.float64)).astype(np.float32)
    Rm, tv = T21[:3, :3], T21[:3, 3]
    txyz = np.einsum('ij,jhw->ihw', Rm, xyz_p).astype(np.float32) \
        + tv[:, None, None].astype(np.float32)
    pos = (txyz[2] > 0).astype(np.float32) * mp

    qa = np.empty((2, CA, cfg.NQ), np.float16)
    ra = np.empty((2, CA, cfg.NQ), np.float16)
    for c in range(3):
        qa[0, c] = _block_q(xyz_p[c], cfg)
        qa[1, c] = _block_q(txyz[c], cfg)
        ra[0, c] = _block_q(xy1[c] * dgt_b, cfg)
        ra[1, c] = _block_q(xy1[c] * dgt_t, cfg)
    # mask channel: (ra3 - qa3)^2 = 0 iff both masks pass, else >= 400
    qa[0, 3] = -MK * (1.0 - _block_q(mp, cfg)).astype(np.float16)
    qa[1, 3] = -MK * (1.0 - _block_q(pos, cfg)).astype(np.float16)
    ra[0, 3] = MK * (1.0 - _block_q(mg_b, cfg)).astype(np.float16)
    ra[1, 3] = MK * (1.0 - _block_q(mg_t, cfg)).astype(np.float16)
    qb = np.stack([_block_q(np.asarray(rgb[b, c], np.float32), cfg)
                   for c in range(3)])
    rbt = np.stack([_block_q(np.asarray(rgb[tb, c], np.float32), cfg)
                    for c in range(3)])
    return {"qa_d": qa, "ra_d": ra, "qb_d": qb, "rbt_d": rbt,
            "selA_d": make_selA(), "selB_d": make_selB()}


def make_in_maps(rgb, depth, depth_gt, depth_mask, depth_gt_mask, xy1_grid, Ts,
                 cfg, n_cores=8):
    return [host_precompute(rgb, depth, depth_gt, depth_mask, depth_gt_mask,
                            xy1_grid, Ts, cfg, b) for b in range(n_cores)]


_CACHED = {}


def _get_nc(cfg_key=(352, 1216, 32)):
    if cfg_key not in _CACHED:
        cfg = Cfg(*cfg_key)
        nc = bass.Bass()
        emit(nc, cfg)
        _CACHED[cfg_key] = (nc, cfg)
    return _CACHED[cfg_key]


def kernel(rgb, depth, depth_gt, depth_mask, depth_gt_mask, xy1_grid, Ts,
           **run_kwargs):
    from concourse.bass_utils import run_bass_kernel_spmd
    nc, cfg = _get_nc()
    maps = make_in_maps(rgb, depth, depth_gt, depth_mask, depth_gt_mask,
                        xy1_grid, Ts, cfg)
    res = run_bass_kernel_spmd(nc, maps, list(range(8)), **run_kwargs)
    total = np.float64(0.0)
    for r in res.results:
        total += np.float64(r["out_d"][:, 0].sum())
    n_gt = max(np.asarray(depth_gt_mask, np.float64).sum(), 1.0)
    loss = -total / n_gt
    kernel.last_results = res
    return np.float32(loss)


# revision 4
# speedup vs baseline: 1.5279x; 1.1004x over previous
"""C3DLoss kernel for Trainium2 — 8-core batch-parallel, raw-Bass implementation.

Per core = one batch frame b (tgt pairing partner tb = b^1):
    partial = sum over both terms (same-frame, cross-frame), all 25 shifts
              delta in [-2,2]^2, all pixels p of
        mref(p) * mq(p+delta) * exp(-50*(|xyz_r(p)-xyz_q(p+d)|^2
                                         + |rgb_r(p)-rgb_q(p+d)|^2))
    loss = -(sum of partials) / max(sum(depth_gt_mask), 1)

v3 design notes:
  - fp16 feature planes; every DVE op is a single contiguous 1-D stream
    (haloed row layout), which is what the DVE 2x_1p fast mode requires
    on HW (strided 2-D access patterns run at 1 elem/cycle).  Halo
    columns ride along in the streams; the PE selector matmul skips them
    with a strided rhs read.
  - Both masks folded into feature channel 3 (ra3=+20*(1-mref),
    qa3=-20*(1-mq)), killing the v1 ref-mask matmul pass.
  - Per-slot work split: DVE does subA, subB (even slots), sqA;
    GpSimd does subB on odd slots; ScalarE squares the rgb diffs in
    slot-pairs and runs exp+accumulate from PSUM.
  - PE runs per 4-slot batch with all selA matmuls consecutive, then all
    selB (2 weight loads per batch instead of 24).
  - Completion signaling via embedded then_inc (no drain bubbles except
    on the relaxed GpSimd stream).
"""

import sys

for _p in ("/opt/trn_rl_repo", "/opt/pypackages"):
    if _p not in sys.path:
        sys.path.insert(0, _p)

from contextlib import ExitStack

import numpy as np

import concourse.bass as bass
import concourse.mybir as mybir
from concourse.ap import AP
from concourse.alu_op_type import AluOpType

F32 = mybir.dt.float32
F16 = mybir.dt.float16

R = 2
G = 32            # W-blocks; one shift-slot = 32 PSUM partitions
CA = 4            # tile A channels: x, y, z, mask
CB = 3            # tile B channels: r, g, b
SBATCH = 4        # shift slots per 128-partition PSUM bank
NPSUM = 8         # rotating PSUM banks (unit = (batch, chunk))
NSQA = 8          # rotating sqa buffers (per slot)
NDP = 4           # rotating db pair buffers (per 2 slots)
NSQB = 4          # rotating sqb pair buffers (per 2 slots)
MK = 20.0         # mask channel scale; (2*MK)^2 = 1600 >> 1/50
EXP_SCALE = -50.0


class Cfg:
    def __init__(self, H=352, W=1216, HS=32):
        assert W % G == 0 and H % HS == 0
        self.H, self.W, self.HS = H, W, HS
        self.WB = W // G                      # 38
        self.WBH = self.WB + 2 * R            # 42
        self.Hp = H + 2 * R                   # 356
        self.NSLAB = H // HS                  # 11
        self.NQ = G * self.Hp * self.WBH      # haloed plane elems
        self.QF = (HS + 2 * R) * self.WBH     # query tile free size 1512
        self.SF = HS * self.WBH               # haloed slot stream size 1344
        # row-chunks of a slot (PSUM bank <= 512 f32/partition)
        cr = 512 // self.WB                   # 13
        self.rchunks = []
        o = 0
        while o < HS:
            self.rchunks.append((o, min(cr, HS - o)))
            o += cr
        self.NC = len(self.rchunks)           # 3
        self.slots = [(t, dy, dx) for t in (0, 1)
                      for dy in range(-R, R + 1) for dx in range(-R, R + 1)]
        self.NS = len(self.slots)             # 50
        assert self.NS % 2 == 0
        self.batches = [self.slots[i:i + SBATCH]
                        for i in range(0, self.NS, SBATCH)]
        self.NB = len(self.batches)           # 13
        self.TOTS = self.NSLAB * self.NS      # 550 slots
        self.TOTB = self.NSLAB * self.NB      # 143 batches
        self.n_acc = self.TOTB * self.NC      # 429 acc columns

    def slot_batch(self, J):
        return (J // self.NS) * self.NB + (J % self.NS) // SBATCH


def _apv(t_ap, p0, pcnt, free_dims, free_off=0):
    pstride = t_ap.ap[0][0]
    base = t_ap.offset + p0 * pstride + free_off
    return AP(t_ap.tensor, base, [[pstride, pcnt]] + [list(d) for d in free_dims])


def _dram_ap(handle, offset, dims):
    a = handle[:]
    return AP(a.tensor, a.offset + offset, [list(d) for d in dims])


def make_selA():
    s = np.zeros((CA * G, G), dtype=np.float16)
    for c in range(CA):
        for g in range(G):
            s[c * G + g, g] = 1
    return s


def make_selB():
    s = np.zeros((CB * G, G), dtype=np.float16)
    for c in range(CB):
        for g in range(G):
            s[c * G + g, g] = 1
    return s


def emit(nc: bass.Bass, cfg: Cfg):
    HS, WB, WBH, Hp = cfg.HS, cfg.WB, cfg.WBH, cfg.Hp
    NQ, QF, SF = cfg.NQ, cfg.QF, cfg.SF
    NSLAB, NB, NC, NS = cfg.NSLAB, cfg.NB, cfg.NC, cfg.NS
    Act = mybir.ActivationFunctionType
    HpW = Hp * WBH

    dp = nc.declare_dram_parameter
    qa_d = dp("qa_d", [2, CA, NQ], F16, isOutput=False)   # query xyz+mask per term
    ra_d = dp("ra_d", [2, CA, NQ], F16, isOutput=False)   # ref xyz+mask per term
    qb_d = dp("qb_d", [CB, NQ], F16, isOutput=False)      # query rgb (frame b)
    rbt_d = dp("rbt_d", [CB, NQ], F16, isOutput=False)    # ref rgb (frame tb)
    selA_d = dp("selA_d", [CA * G, G], F16, isOutput=False)
    selB_d = dp("selB_d", [CB * G, G], F16, isOutput=False)
    out_d = dp("out_d", [128, 1], F32, isOutput=True)

    LD = 6            # load DMAs per slab
    NCONST = 2

    with ExitStack() as ex:
        E = ex.enter_context
        # double-buffered (per slab phase) input planes, haloed layout.
        # q tiles have a +-2 col pad so every shifted 1-D stream stays in
        # bounds; pad cells only ever land on halo columns the PE skips.
        qa_s = [[E(nc.sbuf_tensor(f"qa{t}{p}", [CA * G, QF + 4], F16))
                 for p in range(2)] for t in range(2)]
        ra_s = [[E(nc.sbuf_tensor(f"ra{t}{p}", [CA * G, SF], F16))
                 for p in range(2)] for t in range(2)]
        qb_s = [E(nc.sbuf_tensor(f"qb{p}", [CB * G, QF + 4], F16))
                for p in range(2)]
        rbt_s = [E(nc.sbuf_tensor(f"rbt{p}", [CB * G, SF], F16))
                 for p in range(2)]
        # work tiles (haloed streams)
        da_s = E(nc.sbuf_tensor("da", [CA * G, SF], F16))
        db_s = [E(nc.sbuf_tensor(f"db{i}", [CB * G, 2 * SF], F16))
                for i in range(NDP)]
        sqa_s = [E(nc.sbuf_tensor(f"sqa{i}", [CA * G, SF], F16))
                 for i in range(NSQA)]
        sqb_s = [E(nc.sbuf_tensor(f"sqb{i}", [CB * G, 2 * SF], F16))
                 for i in range(NSQB)]
        kt_s = [E(nc.sbuf_tensor(f"kt{i}", [128, 512], F16)) for i in range(2)]
        acc_s = E(nc.sbuf_tensor("acc", [128, cfg.n_acc], F32))
        res_s = E(nc.sbuf_tensor("res", [128, 1], F32))
        selA_s = E(nc.sbuf_tensor("selA", [CA * G, G], F16))
        selB_s = E(nc.sbuf_tensor("selB", [CB * G, G], F16))
        ps_s = [E(nc.psum_tensor(f"ps{i}", [128, 512], F32))
                for i in range(NPSUM)]

        sL = E(nc.semaphore("sL"))    # final output DMA
        sLC = E(nc.semaphore("sLC"))  # constant loads
        sL0 = E(nc.semaphore("sL0"))  # even-slab loads
        sL1 = E(nc.semaphore("sL1"))  # odd-slab loads
        sG = E(nc.semaphore("sG"))    # gpsimd memset done
        sV = E(nc.semaphore("sV"))    # DVE subB done (even slots; 1/pair)
        sVq = E(nc.semaphore("sVq"))  # DVE sqA done (1/slot) + final reduce
        sPl = E(nc.semaphore("sPl"))  # Pool subB-odd done (1/pair)
        sAq = E(nc.semaphore("sAq"))  # Act sqB pair done (1/pair)
        sP = E(nc.semaphore("sP"))    # PE batch done (1/batch)
        sA = E(nc.semaphore("sA"))    # Act exp units done (1/unit)
        blk = E(nc.Block())

        # access-pattern builders ------------------------------------------
        def q_stream(tile, pcnt, dy, dx):
            # shifted query window as one contiguous stream (pad base +2)
            off = 2 + (R + dy) * WBH + dx
            return _apv(tile.ap(), 0, pcnt, [[1, SF]], off)

        def stream(tile, pcnt, n=SF, off=0):
            return _apv(tile.ap(), 0, pcnt, [[1, n]], off)

        def rgbref_ap(t, ph):
            # ref rgb stream: t=0 reads the query rgb plane at center
            if t == 0:
                return q_stream(qb_s[ph], CB * G, 0, 0)
            return stream(rbt_s[ph], CB * G)

        @blk.gpsimd
        def _(gp):
            gp.memset(acc_s.ap(), 0.0)
            gp.memset(res_s.ap(), 0.0)
            gp.drain()
            gp.sem_inc(sG, 1)
            for s in range(NSLAB):
                ph = s % 2
                sLs = sL0 if ph == 0 else sL1
                gp.wait_ge(sLs, 16 * LD * (s // 2 + 1))
                for j5 in range(1, NS, 2):
                    J = s * NS + j5
                    gpi = J // 2
                    t, dy, dx = cfg.slots[j5]
                    if gpi - NDP + 1 >= 1:
                        gp.wait_ge(sAq, gpi - NDP + 1)
                    nc.gpsimd.tensor_tensor(
                        stream(db_s[gpi % NDP], CB * G, SF, SF),
                        rgbref_ap(t, ph),
                        q_stream(qb_s[ph], CB * G, dy, dx),
                        AluOpType.subtract)
                    gp.drain()
                    gp.sem_inc(sPl, 1)

        @blk.sync
        def _(sp):
            sp.dma_start(selA_s[:], selA_d[:]).then_inc(sLC, 16)
            sp.dma_start(selB_s[:], selB_d[:]).then_inc(sLC, 16)
            for s in range(NSLAB):
                ph = s % 2
                if s >= 2:
                    sp.wait_ge(sVq, NS * (s - 1))
                    sp.wait_ge(sPl, (NS // 2) * (s - 1))
                r0 = s * HS
                sLs = sL0 if ph == 0 else sL1
                for t in range(2):
                    sp.dma_start(
                        _apv(qa_s[t][ph].ap(), 0, CA * G, [[1, QF]], 2),
                        _dram_ap(qa_d, t * CA * NQ + r0 * WBH,
                                 [[NQ, CA], [HpW, G], [1, QF]])
                    ).then_inc(sLs, 16)
                    sp.dma_start(
                        ra_s[t][ph].ap(),
                        _dram_ap(ra_d, t * CA * NQ + (r0 + R) * WBH,
                                 [[NQ, CA], [HpW, G], [1, SF]])
                    ).then_inc(sLs, 16)
                sp.dma_start(
                    _apv(qb_s[ph].ap(), 0, CB * G, [[1, QF]], 2),
                    _dram_ap(qb_d, r0 * WBH, [[NQ, CB], [HpW, G], [1, QF]])
                ).then_inc(sLs, 16)
                sp.dma_start(
                    rbt_s[ph].ap(),
                    _dram_ap(rbt_d, (r0 + R) * WBH,
                             [[NQ, CB], [HpW, G], [1, SF]])
                ).then_inc(sLs, 16)
            sp.wait_ge(sVq, cfg.TOTS + 1)
            sp.dma_start(out_d[:], res_s.ap()).then_inc(sL, 16)

        @blk.vector
        def _(ve):
            for s in range(NSLAB):
                ph = s % 2
                sLs = sL0 if ph == 0 else sL1
                ve.wait_ge(sLs, 16 * LD * (s // 2 + 1))
                for j5 in range(NS):
                    J = s * NS + j5
                    gpi = J // 2
                    t, dy, dx = cfg.slots[j5]
                    if J - NSQA >= 0:
                        ve.wait_ge(sP, cfg.slot_batch(J - NSQA) + 1)
                    if j5 % 2 == 0 and gpi - NDP + 1 >= 1:
                        ve.wait_ge(sAq, gpi - NDP + 1)
                    nc.vector.tensor_tensor(
                        stream(da_s, CA * G),
                        stream(ra_s[t][ph], CA * G),
                        q_stream(qa_s[t][ph], CA * G, dy, dx),
                        AluOpType.subtract)
                    if j5 % 2 == 0:
                        nc.vector.tensor_tensor(
                            stream(db_s[gpi % NDP], CB * G),
                            rgbref_ap(t, ph),
                            q_stream(qb_s[ph], CB * G, dy, dx),
                            AluOpType.subtract).then_inc(sV, 1)
                    nc.vector.tensor_mul(
                        stream(sqa_s[J % NSQA], CA * G),
                        stream(da_s, CA * G),
                        stream(da_s, CA * G)).then_inc(sVq, 1)
            # final reduction of acc columns
            ve.wait_ge(sA, cfg.TOTB * NC)
            nc.vector.tensor_reduce(
                res_s.ap(), acc_s.ap(), axis=mybir.AxisListType.X,
                op=AluOpType.add).then_inc(sVq, 1)

        @blk.tensor
        def _(pe):
            pe.wait_ge(sLC, 16 * NCONST)
            for s in range(NSLAB):
                for b in range(NB):
                    gb = s * NB + b
                    L = len(cfg.batches[b])
                    gJ0 = s * NS + b * SBATCH
                    if NC * gb - NPSUM + NC >= 1:
                        pe.wait_ge(sA, NC * gb - NPSUM + NC)
                    for jj in range(L):
                        J = gJ0 + jj
                        pe.wait_ge(sVq, J + 1)
                        for c, (ro, nr) in enumerate(cfg.rchunks):
                            u = gb * NC + c
                            cn = nr * WB
                            nc.tensor.matmul(
                                ps_s[u % NPSUM][G * jj:G * (jj + 1), :cn],
                                selA_s[:],
                                _apv(sqa_s[J % NSQA].ap(), 0, CA * G,
                                     [[WBH, nr], [1, WB]], ro * WBH + 2),
                                start=True, stop=False, skip_group_check=True,
                                tile_position=(0, G * jj))
                    for jj in range(L):
                        J = gJ0 + jj
                        if jj % 2 == 0:
                            pe.wait_ge(sAq, J // 2 + 1)
                        for c, (ro, nr) in enumerate(cfg.rchunks):
                            u = gb * NC + c
                            cn = nr * WB
                            mm = nc.tensor.matmul(
                                ps_s[u % NPSUM][G * jj:G * (jj + 1), :cn],
                                selB_s[:],
                                _apv(sqb_s[(J // 2) % NSQB].ap(), 0, CB * G,
                                     [[WBH, nr], [1, WB]],
                                     (J % 2) * SF + ro * WBH + 2),
                                start=False, stop=True, skip_group_check=True,
                                tile_position=(0, G * jj))
                            if jj == L - 1 and c == NC - 1:
                                mm.then_inc(sP, 1)

        @blk.scalar
        def _(ac):
            ac.wait_ge(sG, 1)
            for s in range(NSLAB):
                for b in range(NB):
                    gb = s * NB + b
                    L = len(cfg.batches[b])
                    gJ0 = s * NS + b * SBATCH
                    for k in range(L // 2):
                        gpi = gJ0 // 2 + k
                        ac.wait_ge(sV, gpi + 1)
                        ac.wait_ge(sPl, gpi + 1)
                        if gpi - NSQB >= 0:
                            ac.wait_ge(
                                sP, cfg.slot_batch(2 * (gpi - NSQB) + 1) + 1)
                        nc.scalar.activation(
                            stream(sqb_s[gpi % NSQB], CB * G, 2 * SF),
                            stream(db_s[gpi % NDP], CB * G, 2 * SF),
                            Act.Square).then_inc(sAq, 1)
                    pb = G * L
                    ac.wait_ge(sP, gb + 1)
                    for c, (ro, nr) in enumerate(cfg.rchunks):
                        u = gb * NC + c
                        cn = nr * WB
                        nc.scalar.activation(
                            kt_s[u % 2][:pb, :cn], ps_s[u % NPSUM][:pb, :cn],
                            Act.Exp, scale=EXP_SCALE,
                            accum_out=acc_s[:pb, u:u + 1]).then_inc(sA, 1)
    return nc


# ---------------- host side ----------------

def _block_q(plane, cfg):
    """[H, W] -> flat blocked+haloed [G*Hp*WBH] fp16, zero-padded borders."""
    p = np.zeros((cfg.Hp, cfg.W + 2 * R), dtype=np.float32)
    p[R:R + cfg.H, R:R + cfg.W] = plane
    out = np.empty((G, cfg.Hp, cfg.WBH), dtype=np.float16)
    for g in range(G):
        out[g] = p[:, g * cfg.WB:g * cfg.WB + cfg.WBH]
    return np.ascontiguousarray(out).reshape(-1)


def host_precompute(rgb, depth, depth_gt, depth_mask, depth_gt_mask,
                    xy1_grid, Ts, cfg, b):
    tb = b ^ 1
    xy1 = np.asarray(xy1_grid[b], np.float32)
    dep = np.asarray(depth[b, 0], np.float32)
    dgt_b = np.asarray(depth_gt[b, 0], np.float32)
    dgt_t = np.asarray(depth_gt[tb, 0], np.float32)
    mp = np.asarray(depth_mask[b, 0], np.float32)
    mg_b = np.asarray(depth_gt_mask[b, 0], np.float32)
    mg_t = np.asarray(depth_gt_mask[tb, 0], np.float32)

    xyz_p = xy1 * dep
    T21 = (np.linalg.inv(np.asarray(Ts[tb], np.float64)) @
           np.asarray(Ts[b], np.float64)).astype(np.float32)
    Rm, tv = T21[:3, :3], T21[:3, 3]
    txyz = np.einsum('ij,jhw->ihw', Rm, xyz_p).astype(np.float32) \
        + tv[:, None, None].astype(np.float32)
    pos = (txyz[2] > 0).astype(np.float32) * mp

    qa = np.empty((2, CA, cfg.NQ), np.float16)
    ra = np.empty((2, CA, cfg.NQ), np.float16)
    for c in range(3):
        qa[0, c] = _block_q(xyz_p[c], cfg)
        qa[1, c] = _block_q(txyz[c], cfg)
        ra[0, c] = _block_q(xy1[c] * dgt_b, cfg)
        ra[1, c] = _block_q(xy1[c] * dgt_t, cfg)
    # mask channel: (ra3 - qa3)^2 = 0 iff both masks pass, else >= 400
    qa[0, 3] = -MK * (1.0 - _block_q(mp, cfg))
    qa[1, 3] = -MK * (1.0 - _block_q(pos, cfg))
    ra[0, 3] = MK * (1.0 - _block_q(mg_b, cfg))
    ra[1, 3] = MK * (1.0 - _block_q(mg_t, cfg))
    qb = np.stack([_block_q(np.asarray(rgb[b, c], np.float32), cfg)
                   for c in range(3)])
    rbt = np.stack([_block_q(np.asarray(rgb[tb, c], np.float32), cfg)
                    for c in range(3)])
    return {"qa_d": qa, "ra_d": ra, "qb_d": qb, "rbt_d": rbt,
            "selA_d": make_selA(), "selB_d": make_selB()}


def make_in_maps(rgb, depth, depth_gt, depth_mask, depth_gt_mask, xy1_grid, Ts,
                 cfg, n_cores=8):
    return [host_precompute(rgb, depth, depth_gt, depth_mask, depth_gt_mask,
                            xy1_grid, Ts, cfg, b) for b in range(n_cores)]


_CACHED = {}


def _get_nc(cfg_key=(352, 1216, 32)):
    if cfg_key not in _CACHED:
        cfg = Cfg(*cfg_key)
        nc = bass.Bass()
        emit(nc, cfg)
        _CACHED[cfg_key] = (nc, cfg)
    return _CACHED[cfg_key]


def kernel(rgb, depth, depth_gt, depth_mask, depth_gt_mask, xy1_grid, Ts,
           **run_kwargs):
    from concourse.bass_utils import run_bass_kernel_spmd
    nc, cfg = _get_nc()
    maps = make_in_maps(rgb, depth, depth_gt, depth_mask, depth_gt_mask,
                        xy1_grid, Ts, cfg)
    res = run_bass_kernel_spmd(nc, maps, list(range(8)), **run_kwargs)
    total = np.float64(0.0)
    for r in res.results:
        total += np.float64(r["out_d"][:, 0].sum())
    n_gt = max(np.asarray(depth_gt_mask, np.float64).sum(), 1.0)
    loss = -total / n_gt
    kernel.last_results = res
    return np.float32(loss)
